# revision 8
# baseline (speedup 1.0000x reference)
"""Bilinear sampling (nn_Bilinear) Trainium2 Bass kernel.

Full inputs x:[128,224,224,5] f32 (RGB + X + Y sampling coords).
Output: [128,224,224,3] f32 bilinear-sampled images.

Strategy: pure data parallel over 8 NeuronCores (16 batches each).
Per core, the 2D gather runs on the GPSIMD ap_gather instruction over
fp16 tables.  For each batch (one 16-partition Q7 group), 12 table
partitions hold the 3 channel planes x 4 shifted copies (shift 0 / 1 /
224 / 225 of the flat row-major plane).  Tables are addressed at PAIR
granularity (d=2 fp16 = 4B), so one gather index k = floor(l/2)
(l = y0*224+x0) fetches - across the 12 partitions - the pairs
(v[2k+s], v[2k+s+1]) for s in {0,1,224,225}: all four bilinear corners
of all 3 channels, selected by parity of l.  One gather index per
output pixel.  Weights / parity masks / interpolation run on DVE+ACT;
cross-partition data movement bounces through DRAM scratch (SBUF DMA
access patterns only support contiguous partition ranges).
"""
import sys

sys.path.insert(0, "/opt/trn_rl_repo")

import numpy as np

H = 224
NPIX = H * H            # 50176
NB = 128                # full batch
NCORES = 8
BPC = NB // NCORES      # 16 batches per core
WAVES = 2
GW = 8                  # batches per wave (one Q7 group each)
J1 = NPIX // 16         # 3136 pixels per block (Q)
PAIRS = NPIX // 2       # 25088 table elements (d=2)
CF = 112                # chunk width (pixels per block per chunk)
NCH = J1 // CF          # 28 chunks per wave
NI = CF * 16            # 1792 gather indices per Q7 core per instruction
S1W = 7                 # fl = s1*16 + p
QT = 4                  # stage quarters
FQT = J1 // QT          # 784
SHIFTS = (0, 1, 224, 225)
NTAB = (49952, 49952, 49952, 49950)  # table fill lengths per shift
BIG = float(2 ** 23)

_CACHE = {}


def _build_program():
    import concourse.bacc as bacc
    import concourse.tile as tile
    import concourse.mybir as mybir
    from contextlib import ExitStack

    dt = mybir.dt
    Alu = mybir.AluOpType

    nc = bacc.Bacc("TRN2", target_bir_lowering=False, debug=False,
                   enable_asserts=False, num_devices=NCORES)
    x_ap = nc.dram_tensor("x", [BPC, H, H, 5], dt.float32,
                          kind="ExternalInput").ap()
    o_ap = nc.dram_tensor("o", [BPC, H, H, 3], dt.float32,
                          kind="ExternalOutput").ap()

    # [2, 128, 15680]: row (g,q) = batch w*8+g, block q; free (y1 x c)
    x_st = x_ap.rearrange("(w g) (q y) x c -> w (g q) (y x c)", w=WAVES, q=16)
    # [2, 128, 3136, 5] for X/Y extraction
    x_f = x_ap.rearrange("(w g) (q y) x c -> w (g q) (y x) c", w=WAVES, q=16)
    # [2, 128, 9408] output view
    o_st = o_ap.rearrange("(w g) (q y) x c -> w (g q) (y x c)", w=WAVES, q=16)

    with tile.TileContext(nc) as tc:
        with ExitStack() as ctx:
            tabp = ctx.enter_context(tc.tile_pool(name="tab", bufs=1))
            stp = ctx.enter_context(tc.tile_pool(name="st", bufs=1))
            plcp = ctx.enter_context(tc.tile_pool(name="plc", bufs=2))
            wk = ctx.enter_context(tc.tile_pool(name="wk", bufs=2))
            dstp = ctx.enter_context(tc.tile_pool(name="dst", bufs=2))
            cpp = ctx.enter_context(tc.tile_pool(name="cp", bufs=2))
            mp = ctx.enter_context(tc.tile_pool(name="msk", bufs=2))
            outp = ctx.enter_context(tc.tile_pool(name="out", bufs=2))
            drp = ctx.enter_context(
                tc.tile_pool(name="dr", bufs=2, space="DRAM"))
            drp1 = ctx.enter_context(
                tc.tile_pool(name="dr1", bufs=1, space="DRAM"))

            # Gather tables: partition 16g + 4c + si
            TAB = tabp.tile([128, PAIRS, 2], dt.float16)
            TABF = TAB[:].rearrange("p a b -> p (a b)")   # [128, 50176]
            # one-time clear so table tails are finite (parity-masked 0*x)
            nc.vector.memset(TABF, 0.0)

            # DRAM scratch for fp16 planes: row c*8+g -> flat plane
            SCR = drp1.tile([24, NPIX], dt.float16)

            for w in range(WAVES):
                # ---- phase 1: stage -> fp16 planes -> DRAM -> tables ----
                for qt in range(QT):
                    ST = stp.tile([128, FQT * 5], dt.float32, tag="st")
                    nc.sync.dma_start(
                        ST[:], x_st[w][:, qt * FQT * 5:(qt + 1) * FQT * 5])
                    STv = ST[:].rearrange("p (f c) -> p f c", c=5)
                    for c in range(3):
                        PLC = plcp.tile([128, FQT], dt.float16, tag="plc")
                        nc.any.tensor_copy(PLC[:], STv[:, :, c])
                        dst = SCR[c * 8:(c + 1) * 8, :].rearrange(
                            "g (q n) -> (g q) n", n=J1)[:,
                                                        qt * FQT:(qt + 1) * FQT]
                        nc.scalar.dma_start(dst, PLC[:])
                for g in range(GW):
                    for c in range(3):
                        row = c * 8 + g
                        for si in range(4):
                            s, n = SHIFTS[si], NTAB[si]
                            p0 = 16 * g + 4 * c + si
                            nc.sync.dma_start(
                                TABF[p0:p0 + 1, 0:n],
                                SCR[row:row + 1, s:s + n])

                # ---- phase 2: chunks ----
                for ci in range(NCH):
                    fsl = slice(ci * CF, (ci + 1) * CF)

                    Xc = wk.tile([128, CF], dt.float32, tag="xc")
                    Yc = wk.tile([128, CF], dt.float32, tag="yc")
                    nc.sync.dma_start(Xc[:], x_f[w][:, fsl, 3])
                    nc.sync.dma_start(Yc[:], x_f[w][:, fsl, 4])

                    def floor_of(src, tag):
                        t = wk.tile([128, CF], dt.float32, tag=tag + "_t")
                        nc.any.tensor_scalar(t[:], src[:], BIG, -BIG,
                                             Alu.add, Alu.add)
                        m = wk.tile([128, CF], dt.float32, tag=tag + "_m")
                        nc.any.tensor_tensor(m[:], t[:], src[:], Alu.is_gt)
                        f = wk.tile([128, CF], dt.float32, tag=tag)
                        nc.any.tensor_tensor(f[:], t[:], m[:], Alu.subtract)
                        return f

                    fx = floor_of(Xc, "fx")
                    fy = floor_of(Yc, "fy")
                    wx = wk.tile([128, CF], dt.float32, tag="wx")
                    nc.any.tensor_tensor(wx[:], Xc[:], fx[:], Alu.subtract)
                    wy = wk.tile([128, CF], dt.float32, tag="wy")
                    nc.any.tensor_tensor(wy[:], Yc[:], fy[:], Alu.subtract)

                    l = wk.tile([128, CF], dt.float32, tag="l")
                    nc.vector.scalar_tensor_tensor(
                        l[:], in0=fy[:], scalar=float(H), in1=fx[:],
                        op0=Alu.mult, op1=Alu.add)
                    lh = wk.tile([128, CF], dt.float32, tag="lh")
                    nc.any.tensor_scalar_mul(lh[:], l[:], 0.5)
                    kf = floor_of(lh, "kf")
                    par = wk.tile([128, CF], dt.float32, tag="par")
                    nc.vector.scalar_tensor_tensor(
                        par[:], in0=kf[:], scalar=-2.0, in1=l[:],
                        op0=Alu.mult, op1=Alu.add)
                    parp = wk.tile([128, CF], dt.float32, tag="parp")
                    nc.any.tensor_scalar(parp[:], par[:], 0.0, None,
                                         Alu.is_equal)

                    u = wk.tile([128, CF], dt.float32, tag="u")
                    nc.any.tensor_scalar(u[:], wx[:], -1.0, 1.0,
                                         Alu.mult, Alu.add)
                    v = wk.tile([128, CF], dt.float32, tag="v")
                    nc.any.tensor_scalar(v[:], wy[:], -1.0, 1.0,
                                         Alu.mult, Alu.add)
                    wtl = wk.tile([128, CF], dt.float32, tag="wtl")
                    nc.any.tensor_tensor(wtl[:], u[:], v[:], Alu.mult)
                    wtr = wk.tile([128, CF], dt.float32, tag="wtr")
                    nc.any.tensor_tensor(wtr[:], wx[:], v[:], Alu.mult)
                    wbl = wk.tile([128, CF], dt.float32, tag="wbl")
                    nc.any.tensor_tensor(wbl[:], u[:], wy[:], Alu.mult)
                    wbr = wk.tile([128, CF], dt.float32, tag="wbr")
                    nc.any.tensor_tensor(wbr[:], wx[:], wy[:], Alu.mult)

                    # k as int16, in idx-transposed in-partition order
                    # KT[:, p*7+s1] = int16(kf[:, s1*16+p])
                    KT = wk.tile([128, CF], dt.int16, tag="kt")
                    KTv = KT[:].rearrange("n (p s) -> n p s", p=16)
                    kfv = kf[:].rearrange("n (s p) -> n p s", s=S1W)
                    nc.vector.tensor_copy(KTv, kfv)
                    kscr = drp.tile([128, CF], dt.int16, tag="kscr")
                    nc.scalar.dma_start(kscr[:], KT[:])
                    # IDX[16g+p, Q*7+s1] = kscr[16g+Q, p*7+s1]
                    IDX = wk.tile([128, CF], dt.int16, tag="idx")
                    for g in range(GW):
                        src = kscr[g * 16:(g + 1) * 16, :].rearrange(
                            "q (p s) -> p q s", p=16)
                        dst = IDX[g * 16:(g + 1) * 16, :].rearrange(
                            "p (q s) -> p q s", q=16)
                        nc.scalar.dma_start(dst, src)

                    DST = dstp.tile([128, NI, 2], dt.float16, tag="dst")
                    nc.gpsimd.ap_gather(DST[:], TAB[:], IDX[:],
                                        channels=128, num_elems=PAIRS,
                                        d=2, num_idxs=NI)

                    # bounce gather result through DRAM, regroup per batch:
                    # CP[16g+Q, cs, fl, e] = DST[16g+cs, Q*112+fl, e]
                    dscr = drp.tile([128, NI, 2], dt.float16, tag="dscr")
                    nc.sync.dma_start(
                        dscr[:].rearrange("p a b -> p (a b)"),
                        DST[:].rearrange("p a b -> p (a b)"))
                    CP = cpp.tile([128, 12, CF, 2], dt.float16, tag="cp")
                    for g in range(GW):
                        src = dscr[g * 16:g * 16 + 12, :, :].rearrange(
                            "cs (q f) e -> q cs (f e)", q=16)
                        dst = CP[g * 16:(g + 1) * 16, :, :, :].rearrange(
                            "q cs f e -> q cs (f e)")
                        nc.sync.dma_start(dst, src)

                    OUT = outp.tile([128, CF, 3], dt.float32, tag="out")
                    for c in range(3):
                        msks = []
                        for si in range(4):
                            pm = (parp if si in (0, 2) else par)
                            mt = mp.tile([128, CF, 2], dt.float32,
                                         tag=f"m{si}")
                            nc.any.tensor_tensor(
                                mt[:], CP[:, c * 4 + si, :, :],
                                pm[:].unsqueeze(-1).broadcast_to([128, CF, 2]),
                                Alu.mult)
                            msks.append(mt)
                        TL = mp.tile([128, CF], dt.float32, tag="tl")
                        nc.any.tensor_tensor(TL[:], msks[0][:, :, 0],
                                             msks[1][:, :, 0], Alu.add)
                        TR = mp.tile([128, CF], dt.float32, tag="tr")
                        nc.any.tensor_tensor(TR[:], msks[0][:, :, 1],
                                             msks[1][:, :, 1], Alu.add)
                        BL = mp.tile([128, CF], dt.float32, tag="bl")
                        nc.any.tensor_tensor(BL[:], msks[2][:, :, 0],
                                             msks[3][:, :, 0], Alu.add)
                        BR = mp.tile([128, CF], dt.float32, tag="br")
                        nc.any.tensor_tensor(BR[:], msks[2][:, :, 1],
                                             msks[3][:, :, 1], Alu.add)
                        # acc = ((wtl*TL + wbl*BL) + wtr*TR) + wbr*BR
                        a1 = mp.tile([128, CF], dt.float32, tag="a1")
                        nc.any.tensor_tensor(a1[:], wtl[:], TL[:], Alu.mult)
                        a2 = mp.tile([128, CF], dt.float32, tag="a2")
                        nc.any.tensor_tensor(a2[:], wbl[:], BL[:], Alu.mult)
                        nc.any.tensor_tensor(a1[:], a1[:], a2[:], Alu.add)
                        nc.any.tensor_tensor(a2[:], wtr[:], TR[:], Alu.mult)
                        nc.any.tensor_tensor(a1[:], a1[:], a2[:], Alu.add)
                        nc.any.tensor_tensor(a2[:], wbr[:], BR[:], Alu.mult)
                        nc.any.tensor_tensor(OUT[:, :, c], a1[:], a2[:],
                                             Alu.add)

                    nc.sync.dma_start(
                        o_st[w][:, ci * CF * 3:(ci + 1) * CF * 3],
                        OUT[:].rearrange("p a b -> p (a b)"))

    nc.compile()
    return nc


def _get_program():
    if "nc" not in _CACHE:
        _CACHE["nc"] = _build_program()
    return _CACHE["nc"]


def _get_runner():
    """Build (once) a cached jitted shard_map callable over 8 cores.

    Mirrors concourse.bass2jax.run_bass_via_pjrt's multi-core path, but
    keeps the jitted function so repeat calls skip retracing/compiles.
    """
    if "runner" in _CACHE:
        return _CACHE["runner"]
    import jax
    import jax.numpy as jnp  # noqa: F401
    from jax.sharding import Mesh, PartitionSpec
    from jax.experimental.shard_map import shard_map
    import concourse.mybir as mybir
    from concourse import bass2jax

    nc = _get_program()
    bass2jax.install_neuronx_cc_hook()

    partition_name = (nc.partition_id_tensor.name
                      if nc.partition_id_tensor else None)
    in_names, out_names, out_avals, zero_shapes = [], [], [], []
    for alloc in nc.m.functions[0].allocations:
        if not isinstance(alloc, mybir.MemoryLocationSet):
            continue
        name = alloc.memorylocations[0].name
        if alloc.kind == "ExternalInput":
            if name != partition_name:
                in_names.append(name)
        elif alloc.kind == "ExternalOutput":
            shape = tuple(alloc.tensor_shape)
            dtype = mybir.dt.np(alloc.dtype)
            out_names.append(name)
            out_avals.append(jax.core.ShapedArray(shape, dtype))
            zero_shapes.append((shape, dtype))
    n_params = len(in_names)
    n_outs = len(out_avals)
    all_names = list(in_names) + list(out_names)
    if partition_name is not None:
        all_names.append(partition_name)

    def _body(*args):
        operands = list(args)
        if partition_name is not None:
            operands.append(bass2jax.partition_id_tensor())
        outs = bass2jax._bass_exec_p.bind(
            *operands,
            out_avals=tuple(out_avals),
            in_names=tuple(all_names),
            out_names=tuple(out_names),
            lowering_input_output_aliases=(),
            sim_require_finite=True,
            sim_require_nnan=True,
            nc=nc,
        )
        return tuple(outs)

    devices = jax.devices()[:NCORES]
    assert len(devices) == NCORES, f"need {NCORES} devices, got {devices}"
    mesh = Mesh(np.asarray(devices), ("core",))
    in_specs = (PartitionSpec("core"),) * (n_params + n_outs)
    out_specs = (PartitionSpec("core"),) * n_outs
    donate = tuple(range(n_params, n_params + n_outs))
    sharded = jax.jit(
        shard_map(_body, mesh=mesh, in_specs=in_specs, out_specs=out_specs,
                  check_rep=False),
        donate_argnums=donate, keep_unused=True)

    runner = (sharded, in_names, out_names, zero_shapes)
    _CACHE["runner"] = runner
    return runner


def kernel(x: np.ndarray) -> np.ndarray:
    sharded, in_names, out_names, zero_shapes = _get_runner()
    x = np.ascontiguousarray(x, dtype=np.float32)
    assert in_names == ["x"] and out_names == ["o"]
    zeros = [np.zeros((NCORES * s[0], *s[1:]), d) for s, d in zero_shapes]
    out_arrs = sharded(x, *zeros)
    return np.asarray(out_arrs[0])


# revision 15
# speedup vs baseline: 3008.1699x; 3008.1699x over previous
"""Bilinear sampling (nn_Bilinear) Trainium2 Bass kernel.

Full inputs x:[128,224,224,5] f32 (RGB + X + Y sampling coords).
Output: [128,224,224,3] f32 bilinear-sampled images.

Strategy: pure data parallel over 8 NeuronCores (16 batches each).
Per core, the 2D gather runs on the GPSIMD ap_gather instruction over
fp16 tables.  For each batch (one 16-partition Q7 group), 12 table
partitions hold the 3 channel planes x 4 shifted copies (shift 0 / 1 /
224 / 225 of the flat row-major plane).  Tables are addressed at PAIR
granularity (d=2 fp16 = 4B), so one gather index k = floor(l/2)
(l = y0*224+x0) fetches - across the 12 partitions - the pairs
(v[2k+s], v[2k+s+1]) for s in {0,1,224,225}: all four bilinear corners
of all 3 channels, selected by parity of l.  One gather index per
output pixel.  Weights / parity masks / interpolation run on DVE+ACT;
cross-partition regrouping bounces through DRAM scratch (SBUF DMA
access patterns only support contiguous partition ranges).
"""
import sys

sys.path.insert(0, "/opt/trn_rl_repo")

import numpy as np

H = 224
NPIX = H * H            # 50176
NB = 128                # full batch
NCORES = 8
BPC = NB // NCORES      # 16 batches per core
WAVES = 2
GW = 8                  # batches per wave (one Q7 group each)
J1 = NPIX // 16         # 3136 pixels per block (Q)
PAIRS = NPIX // 2       # 25088 table elements (d=2)
CF = 112                # compute sub-chunk width (pixels per block)
CF2 = 224               # gather double-chunk width
NCH2 = J1 // CF2        # 14 double-chunks per wave
NI = CF2 * 16           # 3584 gather indices per Q7 core per instruction
S1W = 7                 # within sub-chunk: fl = s1*16 + p
QT = 8                  # stage slices
FQT = J1 // QT          # 392
SHIFTS = (0, 1, 224, 225)
NTAB = 49952            # table fill length (pairs k <= 24975 used)
SPAD = 256              # leading pad in shifted-plane scratch rows
SROW = SPAD + GW * NPIX + 256   # scratch row length (fp16 elements)
BIG = float(2 ** 23)

_CACHE = {}


def _build_program():
    import concourse.bacc as bacc
    import concourse.tile as tile
    import concourse.mybir as mybir
    from contextlib import ExitStack

    dt = mybir.dt
    Alu = mybir.AluOpType

    nc = bacc.Bacc("TRN2", target_bir_lowering=False, debug=False,
                   enable_asserts=False, num_devices=NCORES)
    x_ap = nc.dram_tensor("x", [BPC, H, H, 5], dt.float32,
                          kind="ExternalInput").ap()
    o_ap = nc.dram_tensor("o", [BPC, H, H, 3], dt.float32,
                          kind="ExternalOutput").ap()

    # [2, 128, 15680]: row (g,q) = batch w*8+g, block q; free (y1 x c)
    x_st = x_ap.rearrange("(w g) (q y) x c -> w (g q) (y x c)", w=WAVES, q=16)
    # [2, 128, 9408] output view
    o_st = o_ap.rearrange("(w g) (q y) x c -> w (g q) (y x c)", w=WAVES, q=16)

    with tile.TileContext(nc) as tc:
        with ExitStack() as ctx:
            tabp = ctx.enter_context(tc.tile_pool(name="tab", bufs=1))
            stp = ctx.enter_context(tc.tile_pool(name="st", bufs=1))
            plcp = ctx.enter_context(tc.tile_pool(name="plc", bufs=1))
            wk = ctx.enter_context(tc.tile_pool(name="wk", bufs=2))
            dstp = ctx.enter_context(tc.tile_pool(name="dst", bufs=2))
            cpp = ctx.enter_context(tc.tile_pool(name="cp", bufs=1))
            mp = ctx.enter_context(tc.tile_pool(name="msk", bufs=1))
            outp = ctx.enter_context(tc.tile_pool(name="out", bufs=2))
            drp = ctx.enter_context(
                tc.tile_pool(name="dr", bufs=2, space="DRAM"))
            drp1 = ctx.enter_context(
                tc.tile_pool(name="dr1", bufs=1, space="DRAM"))
            xyp = ctx.enter_context(
                tc.tile_pool(name="xy", bufs=1, space="DRAM"))

            # Gather tables: partition 16g + 4c + si
            TAB = tabp.tile([128, PAIRS, 2], dt.float16)
            TABF = TAB[:].rearrange("p a b -> p (a b)")   # [128, 50176]
            # one-time clear so table tails are finite (parity-masked 0*x)
            nc.vector.memset(TABF, 0.0)

            # DRAM scratch of pre-shifted fp16 planes: row c*4+si holds the
            # concatenated per-batch planes, each shifted by SHIFTS[si]:
            #   SCR4[c*4+si, SPAD + g*NPIX + j - s] = plane[g][c][j]
            SCR4 = drp1.tile([12, SROW], dt.float16)
            # zero-fill row tails once BEFORE plane writes: the g=7
            # shifted rows leave one stale (masked) slot near the end
            ZT = plcp.tile([128, 768], dt.float16, tag="zt")
            nc.vector.memset(ZT[:], 0.0)
            nc.sync.dma_start(SCR4[:, SROW - 768:SROW], ZT[0:12, :])

            for w in range(WAVES):
                XYS0 = xyp.tile([128, J1], dt.float32, tag="xys0")
                XYS1 = xyp.tile([128, J1], dt.float32, tag="xys1")
                XYS = [XYS0, XYS1]
                # ---- phase 1: stage -> fp16 planes -> DRAM -> tables ----
                for qt in range(QT):
                    ST = stp.tile([128, FQT * 5], dt.float32, tag="st")
                    nc.sync.dma_start(
                        ST[:], x_st[w][:, qt * FQT * 5:(qt + 1) * FQT * 5])
                    STv = ST[:].rearrange("p (f c) -> p f c", c=5)
                    for c in range(3):
                        PLC = plcp.tile([128, FQT], dt.float16, tag="plc")
                        nc.any.tensor_copy(PLC[:], STv[:, :, c])
                        for si in range(4):
                            s = SHIFTS[si]
                            r = c * 4 + si
                            dst = SCR4[r:r + 1,
                                       SPAD - s:SPAD - s + GW * NPIX]\
                                .rearrange("o (p n) -> (o p) n", n=J1)[
                                    :, qt * FQT:(qt + 1) * FQT]
                            eng = nc.scalar if (c + si) % 2 else nc.sync
                            eng.dma_start(dst, PLC[:])
                    for s_i in range(2):
                        XP = plcp.tile([128, FQT], dt.float32, tag="xp")
                        nc.any.tensor_copy(XP[:], STv[:, :, 3 + s_i])
                        nc.sync.dma_start(
                            XYS[s_i][:, qt * FQT:(qt + 1) * FQT], XP[:])
                # table fill: one DMA per (g, c) covering 4 shift partitions
                for g in range(GW):
                    for c in range(3):
                        p0 = 16 * g + 4 * c
                        src = SCR4[c * 4:(c + 1) * 4,
                                   SPAD + g * NPIX:SPAD + g * NPIX + NTAB]
                        eng = nc.scalar if (g + c) % 2 else nc.sync
                        eng.dma_start(TABF[p0:p0 + 4, 0:NTAB], src)

                # ---- phase 2: double-chunks ----
                for ci in range(NCH2):
                    KT2 = wk.tile([128, CF2], dt.int16, tag="kt")
                    pp = {}
                    for u in range(2):
                        fsl = slice(ci * CF2 + u * CF,
                                    ci * CF2 + (u + 1) * CF)

                        Xc = wk.tile([128, CF], dt.float32, tag="xc")
                        Yc = wk.tile([128, CF], dt.float32, tag="yc")
                        nc.sync.dma_start(Xc[:], XYS[0][:, fsl])
                        nc.scalar.dma_start(Yc[:], XYS[1][:, fsl])

                        def floor_of(src, tag):
                            t = wk.tile([128, CF], dt.float32, tag="flo_t")
                            nc.any.tensor_scalar(t[:], src[:], BIG, -BIG,
                                                 Alu.add, Alu.add)
                            m = wk.tile([128, CF], dt.float32, tag="flo_m")
                            nc.any.tensor_tensor(m[:], t[:], src[:],
                                                 Alu.is_gt)
                            f = wk.tile([128, CF], dt.float32, tag=tag)
                            nc.any.tensor_tensor(f[:], t[:], m[:],
                                                 Alu.subtract)
                            return f

                        fx = floor_of(Xc, "fx")
                        fy = floor_of(Yc, "fy")
                        wx = wk.tile([128, CF], dt.float32, tag="wx")
                        nc.any.tensor_tensor(wx[:], Xc[:], fx[:],
                                             Alu.subtract)
                        wy = wk.tile([128, CF], dt.float32, tag="wy")
                        nc.any.tensor_tensor(wy[:], Yc[:], fy[:],
                                             Alu.subtract)

                        l = wk.tile([128, CF], dt.float32, tag="l")
                        nc.vector.scalar_tensor_tensor(
                            l[:], in0=fy[:], scalar=float(H), in1=fx[:],
                            op0=Alu.mult, op1=Alu.add)
                        lh = wk.tile([128, CF], dt.float32, tag="lh")
                        nc.any.tensor_scalar_mul(lh[:], l[:], 0.5)
                        kf = floor_of(lh, "kf")
                        par = wk.tile([128, CF], dt.float32, tag=f"par{u}")
                        nc.vector.scalar_tensor_tensor(
                            par[:], in0=kf[:], scalar=-2.0, in1=l[:],
                            op0=Alu.mult, op1=Alu.add)
                        parp = wk.tile([128, CF], dt.float32, tag=f"parp{u}")
                        nc.any.tensor_scalar(parp[:], par[:], 0.0, None,
                                             Alu.is_equal)

                        uu = wk.tile([128, CF], dt.float32, tag="u")
                        nc.any.tensor_scalar(uu[:], wx[:], -1.0, 1.0,
                                             Alu.mult, Alu.add)
                        vv = wk.tile([128, CF], dt.float32, tag="v")
                        nc.any.tensor_scalar(vv[:], wy[:], -1.0, 1.0,
                                             Alu.mult, Alu.add)
                        wtl = wk.tile([128, CF], dt.float32, tag=f"wtl{u}")
                        nc.any.tensor_tensor(wtl[:], uu[:], vv[:], Alu.mult)
                        wtr = wk.tile([128, CF], dt.float32, tag=f"wtr{u}")
                        nc.any.tensor_tensor(wtr[:], wx[:], vv[:], Alu.mult)
                        wbl = wk.tile([128, CF], dt.float32, tag=f"wbl{u}")
                        nc.any.tensor_tensor(wbl[:], uu[:], wy[:], Alu.mult)
                        wbr = wk.tile([128, CF], dt.float32, tag=f"wbr{u}")
                        nc.any.tensor_tensor(wbr[:], wx[:], wy[:], Alu.mult)
                        pp[u] = (par, parp, wtl, wtr, wbl, wbr)

                        # KT2[:, p*14 + u*7 + s1] = int16(kf[:, s1*16 + p])
                        KTv = KT2[:].rearrange("n (p v s) -> n v p s",
                                               p=16, v=2)[:, u, :, :]
                        kfv = kf[:].rearrange("n (s p) -> n p s", s=S1W)
                        nc.vector.tensor_copy(KTv, kfv)

                    kscr = drp.tile([128, CF2], dt.int16, tag="kscr")
                    nc.scalar.dma_start(kscr[:], KT2[:])
                    # IDX2[16g+p, q*14 + u*7 + s1] = kscr[16g+q, p*14+u*7+s1]
                    IDX2 = wk.tile([128, CF2], dt.int16, tag="idx")
                    for g in range(GW):
                        src = kscr[g * 16:(g + 1) * 16, :].rearrange(
                            "q (p m) -> p q m", p=16)
                        dst = IDX2[g * 16:(g + 1) * 16, :].rearrange(
                            "p (q m) -> p q m", q=16)
                        eng = nc.scalar if g % 2 == 0 else nc.sync
                        eng.dma_start(dst, src)

                    DST = dstp.tile([128, NI, 2], dt.float16, tag="dst")
                    nc.gpsimd.ap_gather(DST[:], TAB[:], IDX2[:],
                                        channels=128, num_elems=PAIRS,
                                        d=2, num_idxs=NI)

                    # bounce gather result through DRAM, regroup per batch:
                    # CP[16g+q, cs, fl2, e] = DST[16g+cs, q*224 + fl2, e]
                    dscr = drp.tile([128, NI, 2], dt.float16, tag="dscr")
                    nc.sync.dma_start(
                        dscr[:].rearrange("p a b -> p (a b)"),
                        DST[:].rearrange("p a b -> p (a b)"))
                    CP = cpp.tile([128, 12, CF2, 2], dt.float16, tag="cp")
                    for g in range(GW):
                        src = dscr[g * 16:g * 16 + 12, :, :].rearrange(
                            "cs (q r) e -> q cs (r e)", q=16)
                        dst = CP[g * 16:(g + 1) * 16, :, :, :].rearrange(
                            "q cs f e -> q cs (f e)")
                        eng = nc.sync if g % 2 else nc.scalar
                        eng.dma_start(dst, src)

                    for u in range(2):
                        par, parp, wtl, wtr, wbl, wbr = pp[u]
                        usl = slice(u * CF, (u + 1) * CF)
                        OUT = outp.tile([128, CF, 3], dt.float32, tag="out")
                        for c in range(3):
                            msks = []
                            for si in range(4):
                                pm = (parp if si in (0, 2) else par)
                                mt = mp.tile([128, CF, 2], dt.float32,
                                             tag=f"m{si}")
                                nc.any.tensor_tensor(
                                    mt[:], CP[:, c * 4 + si, usl, :],
                                    pm[:].unsqueeze(-1).broadcast_to(
                                        [128, CF, 2]),
                                    Alu.mult)
                                msks.append(mt)
                            TL = mp.tile([128, CF], dt.float32, tag="tl")
                            nc.any.tensor_tensor(TL[:], msks[0][:, :, 0],
                                                 msks[1][:, :, 0], Alu.add)
                            TR = mp.tile([128, CF], dt.float32, tag="tr")
                            nc.any.tensor_tensor(TR[:], msks[0][:, :, 1],
                                                 msks[1][:, :, 1], Alu.add)
                            BL = mp.tile([128, CF], dt.float32, tag="bl")
                            nc.any.tensor_tensor(BL[:], msks[2][:, :, 0],
                                                 msks[3][:, :, 0], Alu.add)
                            BR = mp.tile([128, CF], dt.float32, tag="br")
                            nc.any.tensor_tensor(BR[:], msks[2][:, :, 1],
                                                 msks[3][:, :, 1], Alu.add)
                            # acc = ((wtl*TL + wbl*BL) + wtr*TR) + wbr*BR
                            a1 = mp.tile([128, CF], dt.float32, tag="a1")
                            nc.any.tensor_tensor(a1[:], wtl[:], TL[:],
                                                 Alu.mult)
                            a2 = mp.tile([128, CF], dt.float32, tag="a2")
                            nc.any.tensor_tensor(a2[:], wbl[:], BL[:],
                                                 Alu.mult)
                            nc.any.tensor_tensor(a1[:], a1[:], a2[:],
                                                 Alu.add)
                            nc.any.tensor_tensor(a2[:], wtr[:], TR[:],
                                                 Alu.mult)
                            nc.any.tensor_tensor(a1[:], a1[:], a2[:],
                                                 Alu.add)
                            nc.any.tensor_tensor(a2[:], wbr[:], BR[:],
                                                 Alu.mult)
                            nc.any.tensor_tensor(OUT[:, :, c], a1[:], a2[:],
                                                 Alu.add)

                        off = (ci * CF2 + u * CF) * 3
                        eng = nc.sync if u else nc.scalar
                        eng.dma_start(
                            o_st[w][:, off:off + CF * 3],
                            OUT[:].rearrange("p a b -> p (a b)"))

    nc.compile()
    return nc


def _get_program():
    if "nc" not in _CACHE:
        _CACHE["nc"] = _build_program()
    return _CACHE["nc"]


def _get_runner():
    """Build (once) a cached jitted shard_map callable over 8 cores.

    Mirrors concourse.bass2jax.run_bass_via_pjrt's multi-core path, but
    keeps the jitted function so repeat calls skip retracing/compiles.
    """
    if "runner" in _CACHE:
        return _CACHE["runner"]
    import jax
    from jax.sharding import Mesh, PartitionSpec
    from jax.experimental.shard_map import shard_map
    import concourse.mybir as mybir
    from concourse import bass2jax

    nc = _get_program()
    bass2jax.install_neuronx_cc_hook()

    partition_name = (nc.partition_id_tensor.name
                      if nc.partition_id_tensor else None)
    in_names, out_names, out_avals, zero_shapes = [], [], [], []
    for alloc in nc.m.functions[0].allocations:
        if not isinstance(alloc, mybir.MemoryLocationSet):
            continue
        name = alloc.memorylocations[0].name
        if alloc.kind == "ExternalInput":
            if name != partition_name:
                in_names.append(name)
        elif alloc.kind == "ExternalOutput":
            shape = tuple(alloc.tensor_shape)
            dtype = mybir.dt.np(alloc.dtype)
            out_names.append(name)
            out_avals.append(jax.core.ShapedArray(shape, dtype))
            zero_shapes.append((shape, dtype))
    n_params = len(in_names)
    n_outs = len(out_avals)
    all_names = list(in_names) + list(out_names)
    if partition_name is not None:
        all_names.append(partition_name)

    def _body(*args):
        operands = list(args)
        if partition_name is not None:
            operands.append(bass2jax.partition_id_tensor())
        outs = bass2jax._bass_exec_p.bind(
            *operands,
            out_avals=tuple(out_avals),
            in_names=tuple(all_names),
            out_names=tuple(out_names),
            lowering_input_output_aliases=(),
            sim_require_finite=True,
            sim_require_nnan=True,
            nc=nc,
        )
        return tuple(outs)

    devices = jax.devices()[:NCORES]
    assert len(devices) == NCORES, f"need {NCORES} devices, got {devices}"
    mesh = Mesh(np.asarray(devices), ("core",))
    in_specs = (PartitionSpec("core"),) * (n_params + n_outs)
    out_specs = (PartitionSpec("core"),) * n_outs
    donate = tuple(range(n_params, n_params + n_outs))
    sharded = jax.jit(
        shard_map(_body, mesh=mesh, in_specs=in_specs, out_specs=out_specs,
                  check_rep=False),
        donate_argnums=donate, keep_unused=True)

    runner = (sharded, in_names, out_names, zero_shapes)
    _CACHE["runner"] = runner
    return runner


def kernel(x: np.ndarray) -> np.ndarray:
    sharded, in_names, out_names, zero_shapes = _get_runner()
    x = np.ascontiguousarray(x, dtype=np.float32)
    assert in_names == ["x"] and out_names == ["o"]
    zeros = [np.zeros((NCORES * s[0], *s[1:]), d) for s, d in zero_shapes]
    out_arrs = sharded(x, *zeros)
    return np.asarray(out_arrs[0])
